# revision 40
# baseline (speedup 1.0000x reference)
"""Trainium2 Bass kernel for 3D-PoPE multi-head self-attention.

Sharding: pure data-parallel over batch (B=8 -> 8 cores, one batch element
per core). Weights replicated. All rotary/cache gathers precomputed on host
(tiny tensors); all matmuls/softmax on device.

Per-core computation (S=1024, D=1024, H=16, HD=64), single fused pipeline
so the PE-bound projection work overlaps the ACT-bound attention exp work:

  - v-projection st-tiles interleaved with heads 0/1 (PV of k-tile kt only
    needs v rows kt, so attention starts before v finishes)
  - then per j in 1..7: [q-proj j] [head 2j] [k-proj j] [head 2j+1]
  - one PSUM tag (2 slots, 4 banks) shared by projection and score tiles;
    pv accumulators double-buffered (4 banks)
  - softplus = Ln(Exp(x)+1); one explicit preload of the combined
    natural_log_exp_and_others ACT table set serves Exp, Ln, and the
    attention Exp with zero table switches
  - softmax denominator rides PV as a ones-row (row 64); normalization is
    reciprocal_approx_fast + gpsimd partition_broadcast + one DVE mul,
    entirely off the PE critical path
  - out = attnT.T @ w_out.T
"""
import math

import numpy as np
import ml_dtypes

B, S, D, H = 8, 1024, 1024, 16
HD = D // H
DX = HD // 3
DY = HD // 3
DZ = HD - DX - DY
MX, MY, MZ = 32, 32, 8
BASE = 10000.0
TWO_PI = 2.0 * math.pi
BF16 = ml_dtypes.bfloat16

NT = S // 128          # 8 sequence tiles
ND = D // 128          # 8 contraction tiles
SCALE = 1.0 / math.sqrt(2.0 * HD)


def _host_prep(hidden_states, pos_xyz, w_qkv, w_out, phase_bias):
    """Host-side: transposes, dtype casts, rotary cache gather."""
    def cache(dim, maxp):
        inv = 1.0 / (BASE ** (np.arange(dim, dtype=np.float64) / dim))
        t = np.arange(maxp, dtype=np.float64)[:, None] * inv[None, :]
        return np.cos(t), np.sin(t)

    cx, sx = cache(DX, MX)
    cy, sy = cache(DY, MY)
    cz, sz = cache(DZ, MZ)
    pos = np.asarray(pos_xyz)
    px = np.clip(pos[..., 0], 0, MX - 1).astype(np.int64)
    py = np.clip(pos[..., 1], 0, MY - 1).astype(np.int64)
    pz = np.clip(pos[..., 2], 0, MZ - 1).astype(np.int64)
    cos_t = np.concatenate([cx[px], cy[py], cz[pz]], axis=-1)  # [B,S,HD] f64
    sin_t = np.concatenate([sx[px], sy[py], sz[pz]], axis=-1)
    bias = np.clip(np.asarray(phase_bias, np.float64), -TWO_PI, 0.0)
    cos_b = np.cos(bias)
    sin_b = np.sin(bias)
    cos_k = cos_t * cos_b - sin_t * sin_b
    sin_k = sin_t * cos_b + cos_t * sin_b

    def dup(x):
        # [B,S,HD] -> [B, 128, S] bf16 with rows 0:64 == rows 64:128
        xt = np.ascontiguousarray(x.transpose(0, 2, 1))  # [B, HD, S]
        return np.concatenate([xt, xt], axis=1).astype(BF16)

    hs = np.asarray(hidden_states, np.float32)
    hsT = np.ascontiguousarray(hs.transpose(0, 2, 1)).astype(BF16)  # [B, D, S]
    wqkvT = np.ascontiguousarray(np.asarray(w_qkv, np.float32).T).astype(BF16)
    woutT = np.ascontiguousarray(np.asarray(w_out, np.float32).T).astype(BF16)
    return hsT, wqkvT, woutT, dup(cos_t), dup(sin_t), dup(cos_k), dup(sin_k)


def _emit(tc, nc, t_hsT, t_wqkvT, t_woutT, t_rot, t_out):
    import concourse.mybir as mybir

    dt = mybir.dt
    AF = mybir.ActivationFunctionType
    f32 = dt.float32
    bf = dt.bfloat16

    tables = None
    try:
        from concourse.hw_specs import get_activation_tables
        tables = list(get_activation_tables(nc.m.arch).keys())
    except Exception:
        pass
    if tables and "natural_log_exp_and_others" in tables:
        nc.scalar.add_instruction(mybir.InstLoadActFuncSet(
            act_func_set_id=tables.index("natural_log_exp_and_others"),
            name=nc.get_next_instruction_name(), ins=[], outs=[]))

    with (
        tc.tile_pool(name="persist", bufs=1) as pp,
        tc.tile_pool(name="work", bufs=1) as wp,
    ):
        psm_cm = tc.tile_pool(name="psum_main", bufs=1, space="PSUM")
        psm = psm_cm.__enter__()
        vaug = pp.tile([128, NT * H * 65], bf, tag="vaug", bufs=1)
        va_r = vaug.rearrange("p (k h c) -> p k h c", k=NT, h=H)
        nc.vector.memset(va_r[:, :, :, 64:65], 1.0)
        attnT = pp.tile([128, 8 * S], bf, tag="attnT", bufs=1)

        def big():
            return psm.tile([128, S], f32, tag="big", bufs=2, name="big")

        # ---- input DMA: v-projection runs first, so interleave hsT/wv
        # blocks up front; wq/wk/rot follow behind compute.
        hsT = wp.tile([128, ND * S], bf, tag="hsT", bufs=1)
        wv = wp.tile([128, ND * D], bf, tag="w", bufs=3)
        # wv rides gpsimd's otherwise-idle DGE queues so the 4MB of
        # v-projection input streams in parallel with hsT on sync's
        for i in range(ND):
            nc.sync.dma_start(
                hsT[:, i * S:(i + 1) * S], t_hsT[i * 128:(i + 1) * 128, :])
            nc.gpsimd.dma_start(
                wv[:, i * D:(i + 1) * D],
                t_wqkvT[i * 128:(i + 1) * 128, 2 * D:3 * D])

        def load_w(col0):
            w = wp.tile([128, ND * D], bf, tag="w", bufs=3)
            for i in range(ND):
                nc.sync.dma_start(
                    w[:, i * D:(i + 1) * D],
                    t_wqkvT[i * 128:(i + 1) * 128, col0:col0 + D])
            return w

        def load_w_out():
            w = wp.tile([128, ND * D], bf, tag="w", bufs=3, name="wo")
            for i in range(ND):
                nc.sync.dma_start(
                    w[:, i * D:(i + 1) * D], t_woutT[i * 128:(i + 1) * 128, :])
            return w

        wq = load_w(0)
        wk = load_w(D)
        rot = wp.tile([128, 4 * S], bf, tag="rot", bufs=1)
        nc.sync.dma_start(rot[:], t_rot[:])
        r_cq = rot[:, 0:S]
        r_sq = rot[:, S:2 * S]
        r_ck = rot[:, 2 * S:3 * S]
        r_sk = rot[:, 3 * S:4 * S]

        def proj_mm(is_k, j, ps, di):
            w = wk if is_k else wq
            lhsT = w[:, di * D + j * 128:di * D + (j + 1) * 128]
            rhs = hsT[:, di * S:(di + 1) * S]
            nc.tensor.matmul(ps[:, 0:512], lhsT, rhs[:, 0:512],
                             start=(di == 0), stop=(di == ND - 1))
            nc.tensor.matmul(ps[:, 512:1024], lhsT, rhs[:, 512:1024],
                             start=(di == 0), stop=(di == ND - 1))

        def proj_ps():
            return psm.tile([128, S], f32, tag="pproj", bufs=1, name="pproj")

        def proj_finish(is_k, j, ps):
            """Softplus + PoPE once the 16 projection matmuls are in."""
            pse = wp.tile([128, S], bf, tag="pse", bufs=2)
            nc.scalar.activation(pse[:], ps[:], AF.Exp)
            mu = wp.tile([128, S], bf, tag="mu", bufs=2)
            nc.scalar.activation(mu[:], pse[:], AF.Ln, bias=1.0)
            dest = wp.tile([128, 2 * S], bf, tag="k2" if is_k else "q2",
                           bufs=3, name="k2" if is_k else "q2")
            cosr = r_ck if is_k else r_cq
            sinr = r_sk if is_k else r_sq
            for hh in range(2):
                lo, hi = hh * 64, hh * 64 + 64
                dsl = dest[:, hh * S:(hh + 1) * S]
                nc.vector.tensor_mul(dsl[0:64, :], mu[lo:hi, :], cosr[lo:hi, :])
                nc.vector.tensor_mul(dsl[64:128, :], mu[lo:hi, :],
                                     sinr[lo:hi, :])
            return dest

        def proj_v(st):
            """v-projection rows [st*128, st*128+128) -> vaug columns."""
            ps = big()
            for di in range(ND):
                lhsT = hsT[:, di * S + st * 128:di * S + (st + 1) * 128]
                rhs = wv[:, di * D:(di + 1) * D]
                nc.tensor.matmul(ps[:, 0:512], lhsT, rhs[:, 0:512],
                                 start=(di == 0), stop=(di == ND - 1))
                nc.tensor.matmul(ps[:, 512:1024], lhsT, rhs[:, 512:1024],
                                 start=(di == 0), stop=(di == ND - 1))
            vr = va_r[:, st]
            nc.vector.tensor_copy(
                vr[:, :, 0:64], ps.rearrange("p (h c) -> p h c", c=64))

        def emit_scores(q2t, k2t, hh, kt):
            pss = big()
            q2 = q2t[:, hh * S:(hh + 1) * S]
            k2s = k2t[:, hh * S + kt * 128:hh * S + (kt + 1) * 128]
            nc.tensor.matmul(pss[:, 0:512], k2s, q2[:, 0:512],
                             start=True, stop=True)
            nc.tensor.matmul(pss[:, 512:1024], k2s, q2[:, 512:1024],
                             start=True, stop=True)
            return pss

        def emit_exp_pv(h, kt, pss, pv):
            pt = wp.tile([128, S], bf, tag="pt", bufs=6)
            nc.scalar.activation(pt[:], pss[:], AF.Exp, scale=SCALE)
            va = vaug[:, kt * H * 65 + h * 65:kt * H * 65 + (h + 1) * 65]
            nc.tensor.matmul(pv[:, 0:512], va, pt[:, 0:512],
                             start=(kt == 0), stop=(kt == NT - 1))
            nc.tensor.matmul(pv[:, 512:1024], va, pt[:, 512:1024],
                             start=(kt == 0), stop=(kt == NT - 1))

        def emit_norm(h, pv):
            # Evacuate PV promptly (pv is single-buffered): unnormalized
            # attn rows straight into the attnT slice, denominator row to
            # partition 0 (custom-DVE recip needs aligned partitions).
            # Then normalize the slice in place once the broadcast lands.
            dsl = attnT[(h % 2) * 64:(h % 2) * 64 + 64,
                        (h // 2) * S:(h // 2 + 1) * S]
            nc.vector.tensor_copy(dsl, pv[0:64, :])
            dn = wp.tile([1, S], f32, tag="dn", bufs=1)
            nc.vector.tensor_copy(dn[:], pv[64:65, :])
            rc = wp.tile([1, S], f32, tag="rc", bufs=1)
            nc.vector.reciprocal_approx_fast(rc[:], dn[:])
            bc = wp.tile([128, S], f32, tag="bc", bufs=3)
            nc.gpsimd.partition_broadcast(bc[:], rc[:], channels=128)
            nc.vector.tensor_mul(dsl, dsl,
                                 bc[(h % 2) * 64:(h % 2) * 64 + 64, :])

        def head_full(q2t, k2t, hh, h, inject=None):
            """Full 8-kt attention pipeline for one head. `inject(kt)`
            emits interleaved work (next block's projection matmuls) so
            ACT never waits for a projection burst at group boundaries."""
            pv = psm.tile([65, S], f32, tag="pv", bufs=1, name="pv")
            window = [emit_scores(q2t, k2t, hh, 0), emit_scores(q2t, k2t, hh, 1)]
            for kt in range(NT):
                pss = window.pop(0)
                emit_exp_pv(h, kt, pss, pv)
                if kt + 2 < NT:
                    window.append(emit_scores(q2t, k2t, hh, kt + 2))
                if inject is not None:
                    inject(kt)
            emit_norm(h, pv)

        # ---- fused pipeline. Heads 2g/2g+1 run while block g+1's
        # projection matmuls trickle through their kt loops (2 per step),
        # so softplus input is ready the moment the head finishes and the
        # ACT queue never drains at a group boundary.
        for st in range(NT):
            proj_v(st)
        # wout prefetch: rotates into wv's slot, needed ~200us later
        wo = load_w_out()

        ps = proj_ps()
        for di in range(ND):
            proj_mm(False, 0, ps, di)
        q2p = proj_finish(False, 0, ps)
        ps = proj_ps()
        for di in range(ND):
            proj_mm(True, 0, ps, di)
        k2p = proj_finish(True, 0, ps)

        for g in range(8):
            if g < 7:
                psq = proj_ps()
                head_full(q2p, k2p, 0, 2 * g,
                          inject=lambda kt: proj_mm(False, g + 1, psq, kt))
                q2n = proj_finish(False, g + 1, psq)
                psk = proj_ps()
                head_full(q2p, k2p, 1, 2 * g + 1,
                          inject=lambda kt: proj_mm(True, g + 1, psk, kt))
                k2n = proj_finish(True, g + 1, psk)
                q2p, k2p = q2n, k2n
            else:
                head_full(q2p, k2p, 0, 14)
                head_full(q2p, k2p, 1, 15)

        psm_cm.__exit__(None, None, None)

        # ---------------- output projection ----------------
        with tc.tile_pool(name="psum_tail", bufs=1, space="PSUM") as pst:
            for st in range(NT):
                ps = pst.tile([128, D], f32, tag="ps_o", bufs=2)
                for et in range(ND):
                    lhsT = attnT[:, et * S + st * 128:et * S + (st + 1) * 128]
                    rhs = wo[:, et * D:(et + 1) * D]
                    nc.tensor.matmul(ps[:, 0:512], lhsT, rhs[:, 0:512],
                                     start=(et == 0), stop=(et == ND - 1))
                    nc.tensor.matmul(ps[:, 512:1024], lhsT, rhs[:, 512:1024],
                                     start=(et == 0), stop=(et == ND - 1))
                ot = wp.tile([128, D], f32, tag="out_sb", bufs=2)
                nc.vector.tensor_copy(ot[:], ps[:])
                nc.sync.dma_start(t_out[st * 128:(st + 1) * 128, :], ot[:])


def build_bass(reps=1, **emit_kw):
    import concourse.bass as bass  # noqa: F401
    import concourse.mybir as mybir
    import concourse.tile as tile
    from concourse import bacc

    dt = mybir.dt
    nc = bacc.Bacc("TRN2", target_bir_lowering=False, debug=False)
    t_hsT = nc.dram_tensor("hsT", [D, S], dt.bfloat16, kind="ExternalInput").ap()
    t_wqkvT = nc.dram_tensor("wqkvT", [D, 3 * D], dt.bfloat16, kind="ExternalInput").ap()
    t_woutT = nc.dram_tensor("woutT", [D, D], dt.bfloat16, kind="ExternalInput").ap()
    t_rot = nc.dram_tensor("rot", [128, 4 * S], dt.bfloat16, kind="ExternalInput").ap()
    t_out = nc.dram_tensor("out", [S, D], dt.float32, kind="ExternalOutput").ap()
    with tile.TileContext(nc) as tc:
        if reps == 1:
            _emit(tc, nc, t_hsT, t_wqkvT, t_woutT, t_rot, t_out, **emit_kw)
        else:
            with tc.For_i(0, reps, 1):
                _emit(tc, nc, t_hsT, t_wqkvT, t_woutT, t_rot, t_out, **emit_kw)
    nc.compile()
    return nc


def make_in_maps(hidden_states, pos_xyz, w_qkv, w_out, phase_bias):
    hsT, wqkvT, woutT, cq, sq, ck, sk = _host_prep(
        hidden_states, pos_xyz, w_qkv, w_out, phase_bias)
    rot = np.concatenate([cq, sq, ck, sk], axis=2)  # [B, 128, 4*S]
    return [
        {
            "hsT": np.ascontiguousarray(hsT[b]),
            "wqkvT": wqkvT,
            "woutT": woutT,
            "rot": np.ascontiguousarray(rot[b]),
        }
        for b in range(B)
    ]


def kernel(hidden_states, attention_mask, pos_xyz, w_qkv, w_out, phase_bias):
    from concourse.bass_utils import run_bass_kernel_spmd

    in_maps = make_in_maps(hidden_states, pos_xyz, w_qkv, w_out, phase_bias)
    nc = build_bass()
    res = run_bass_kernel_spmd(nc, in_maps, list(range(B)))
    out = np.stack([np.asarray(res.results[c]["out"]) for c in range(B)])
    return out.astype(np.float32)


# revision 43
# speedup vs baseline: 1.0083x; 1.0083x over previous
"""Trainium2 Bass kernel for 3D-PoPE multi-head self-attention.

Sharding: pure data-parallel over batch (B=8 -> 8 cores, one batch element
per core). Weights replicated. All rotary/cache gathers precomputed on host
(tiny tensors); all matmuls/softmax on device.

Per-core computation (S=1024, D=1024, H=16, HD=64), single fused pipeline
so the PE-bound projection work overlaps the ACT-bound attention exp work:

  - v-projection st-tiles interleaved with heads 0/1 (PV of k-tile kt only
    needs v rows kt, so attention starts before v finishes)
  - then per j in 1..7: [q-proj j] [head 2j] [k-proj j] [head 2j+1]
  - one PSUM tag (2 slots, 4 banks) shared by projection and score tiles;
    pv accumulators double-buffered (4 banks)
  - softplus = Ln(Exp(x)+1); one explicit preload of the combined
    natural_log_exp_and_others ACT table set serves Exp, Ln, and the
    attention Exp with zero table switches
  - softmax denominator rides PV as a ones-row (row 64); normalization is
    reciprocal_approx_fast + gpsimd partition_broadcast + one DVE mul,
    entirely off the PE critical path
  - out = attnT.T @ w_out.T
"""
import math

import numpy as np
import ml_dtypes

B, S, D, H = 8, 1024, 1024, 16
HD = D // H
DX = HD // 3
DY = HD // 3
DZ = HD - DX - DY
MX, MY, MZ = 32, 32, 8
BASE = 10000.0
TWO_PI = 2.0 * math.pi
BF16 = ml_dtypes.bfloat16

NT = S // 128          # 8 sequence tiles
ND = D // 128          # 8 contraction tiles
SCALE = 1.0 / math.sqrt(2.0 * HD)


def _host_prep(hidden_states, pos_xyz, w_qkv, w_out, phase_bias):
    """Host-side: transposes, dtype casts, rotary cache gather."""
    def cache(dim, maxp):
        inv = 1.0 / (BASE ** (np.arange(dim, dtype=np.float64) / dim))
        t = np.arange(maxp, dtype=np.float64)[:, None] * inv[None, :]
        return np.cos(t), np.sin(t)

    cx, sx = cache(DX, MX)
    cy, sy = cache(DY, MY)
    cz, sz = cache(DZ, MZ)
    pos = np.asarray(pos_xyz)
    px = np.clip(pos[..., 0], 0, MX - 1).astype(np.int64)
    py = np.clip(pos[..., 1], 0, MY - 1).astype(np.int64)
    pz = np.clip(pos[..., 2], 0, MZ - 1).astype(np.int64)
    cos_t = np.concatenate([cx[px], cy[py], cz[pz]], axis=-1)  # [B,S,HD] f64
    sin_t = np.concatenate([sx[px], sy[py], sz[pz]], axis=-1)
    bias = np.clip(np.asarray(phase_bias, np.float64), -TWO_PI, 0.0)
    cos_b = np.cos(bias)
    sin_b = np.sin(bias)
    cos_k = cos_t * cos_b - sin_t * sin_b
    sin_k = sin_t * cos_b + cos_t * sin_b

    def dup(x):
        # [B,S,HD] -> [B, 128, S] bf16 with rows 0:64 == rows 64:128
        xt = np.ascontiguousarray(x.transpose(0, 2, 1))  # [B, HD, S]
        return np.concatenate([xt, xt], axis=1).astype(BF16)

    hs = np.asarray(hidden_states, np.float32)
    hsT = np.ascontiguousarray(hs.transpose(0, 2, 1)).astype(BF16)  # [B, D, S]
    wqkvT = np.ascontiguousarray(np.asarray(w_qkv, np.float32).T).astype(BF16)
    woutT = np.ascontiguousarray(np.asarray(w_out, np.float32).T).astype(BF16)
    return hsT, wqkvT, woutT, dup(cos_t), dup(sin_t), dup(cos_k), dup(sin_k)


def _emit(tc, nc, t_hsT, t_wqkvT, t_woutT, t_rot, t_out):
    import concourse.mybir as mybir

    dt = mybir.dt
    AF = mybir.ActivationFunctionType
    f32 = dt.float32
    bf = dt.bfloat16

    tables = None
    try:
        from concourse.hw_specs import get_activation_tables
        tables = list(get_activation_tables(nc.m.arch).keys())
    except Exception:
        pass
    if tables and "natural_log_exp_and_others" in tables:
        nc.scalar.add_instruction(mybir.InstLoadActFuncSet(
            act_func_set_id=tables.index("natural_log_exp_and_others"),
            name=nc.get_next_instruction_name(), ins=[], outs=[]))

    with (
        tc.tile_pool(name="persist", bufs=1) as pp,
        tc.tile_pool(name="work", bufs=1) as wp,
    ):
        psm_cm = tc.tile_pool(name="psum_main", bufs=1, space="PSUM")
        psm = psm_cm.__enter__()
        vaug = pp.tile([128, NT * H * 65], bf, tag="vaug", bufs=1)
        va_r = vaug.rearrange("p (k h c) -> p k h c", k=NT, h=H)
        nc.vector.memset(va_r[:, :, :, 64:65], 1.0)
        attnT = pp.tile([128, 8 * S], bf, tag="attnT", bufs=1)

        def big():
            return psm.tile([128, S], f32, tag="big", bufs=2, name="big")

        # ---- input DMA: v-projection runs first, so interleave hsT/wv
        # blocks up front; wq/wk/rot follow behind compute.
        hsT = wp.tile([128, ND * S], bf, tag="hsT", bufs=1)
        wv = wp.tile([128, ND * D], bf, tag="w", bufs=3)
        # wv rides gpsimd's otherwise-idle DGE queues so the 4MB of
        # v-projection input streams in parallel with hsT on sync's
        for i in range(ND):
            nc.sync.dma_start(
                hsT[:, i * S:(i + 1) * S], t_hsT[i * 128:(i + 1) * 128, :])
            nc.gpsimd.dma_start(
                wv[:, i * D:(i + 1) * D],
                t_wqkvT[i * 128:(i + 1) * 128, 2 * D:3 * D])

        def load_w(col0):
            w = wp.tile([128, ND * D], bf, tag="w", bufs=3)
            for i in range(ND):
                nc.sync.dma_start(
                    w[:, i * D:(i + 1) * D],
                    t_wqkvT[i * 128:(i + 1) * 128, col0:col0 + D])
            return w

        def load_w_out():
            w = wp.tile([128, ND * D], bf, tag="w", bufs=3, name="wo")
            for i in range(ND):
                nc.sync.dma_start(
                    w[:, i * D:(i + 1) * D], t_woutT[i * 128:(i + 1) * 128, :])
            return w

        wq = load_w(0)
        wk = load_w(D)
        rot = wp.tile([128, 4 * S], bf, tag="rot", bufs=1)
        nc.sync.dma_start(rot[:], t_rot[:])
        r_cq = rot[:, 0:S]
        r_sq = rot[:, S:2 * S]
        r_ck = rot[:, 2 * S:3 * S]
        r_sk = rot[:, 3 * S:4 * S]

        def proj_mm(is_k, j, ps, di):
            w = wk if is_k else wq
            lhsT = w[:, di * D + j * 128:di * D + (j + 1) * 128]
            rhs = hsT[:, di * S:(di + 1) * S]
            nc.tensor.matmul(ps[:, 0:512], lhsT, rhs[:, 0:512],
                             start=(di == 0), stop=(di == ND - 1))
            nc.tensor.matmul(ps[:, 512:1024], lhsT, rhs[:, 512:1024],
                             start=(di == 0), stop=(di == ND - 1))

        def proj_ps():
            return psm.tile([128, S], f32, tag="pproj", bufs=1, name="pproj")

        def proj_finish(is_k, j, ps):
            """Softplus + PoPE once the 16 projection matmuls are in."""
            pse = wp.tile([128, S], bf, tag="pse", bufs=2)
            nc.scalar.activation(pse[:], ps[:], AF.Exp)
            mu = wp.tile([128, S], bf, tag="mu", bufs=2)
            nc.scalar.activation(mu[:], pse[:], AF.Ln, bias=1.0)
            dest = wp.tile([128, 2 * S], bf, tag="k2" if is_k else "q2",
                           bufs=3, name="k2" if is_k else "q2")
            cosr = r_ck if is_k else r_cq
            sinr = r_sk if is_k else r_sq
            for hh in range(2):
                lo, hi = hh * 64, hh * 64 + 64
                dsl = dest[:, hh * S:(hh + 1) * S]
                nc.vector.tensor_mul(dsl[0:64, :], mu[lo:hi, :], cosr[lo:hi, :])
                nc.vector.tensor_mul(dsl[64:128, :], mu[lo:hi, :],
                                     sinr[lo:hi, :])
            return dest

        def proj_v(st):
            """v-projection rows [st*128, st*128+128) -> vaug columns."""
            ps = big()
            for di in range(ND):
                lhsT = hsT[:, di * S + st * 128:di * S + (st + 1) * 128]
                rhs = wv[:, di * D:(di + 1) * D]
                nc.tensor.matmul(ps[:, 0:512], lhsT, rhs[:, 0:512],
                                 start=(di == 0), stop=(di == ND - 1))
                nc.tensor.matmul(ps[:, 512:1024], lhsT, rhs[:, 512:1024],
                                 start=(di == 0), stop=(di == ND - 1))
            vr = va_r[:, st]
            nc.vector.tensor_copy(
                vr[:, :, 0:64], ps.rearrange("p (h c) -> p h c", c=64))

        def emit_scores(q2t, k2t, hh, kt):
            pss = big()
            q2 = q2t[:, hh * S:(hh + 1) * S]
            k2s = k2t[:, hh * S + kt * 128:hh * S + (kt + 1) * 128]
            nc.tensor.matmul(pss[:, 0:512], k2s, q2[:, 0:512],
                             start=True, stop=True)
            nc.tensor.matmul(pss[:, 512:1024], k2s, q2[:, 512:1024],
                             start=True, stop=True)
            return pss

        def emit_exp_pv(h, kt, pss, pv):
            pt = wp.tile([128, S], bf, tag="pt", bufs=6)
            nc.scalar.activation(pt[:], pss[:], AF.Exp, scale=SCALE)
            va = vaug[:, kt * H * 65 + h * 65:kt * H * 65 + (h + 1) * 65]
            nc.tensor.matmul(pv[:, 0:512], va, pt[:, 0:512],
                             start=(kt == 0), stop=(kt == NT - 1))
            nc.tensor.matmul(pv[:, 512:1024], va, pt[:, 512:1024],
                             start=(kt == 0), stop=(kt == NT - 1))

        def emit_norm(h, pv):
            # Evacuate PV promptly (pv is single-buffered): unnormalized
            # attn rows straight into the attnT slice, denominator row to
            # partition 0 (custom-DVE recip needs aligned partitions).
            # Then normalize the slice in place once the broadcast lands.
            dsl = attnT[(h % 2) * 64:(h % 2) * 64 + 64,
                        (h // 2) * S:(h // 2 + 1) * S]
            nc.vector.tensor_copy(dsl, pv[0:64, :])
            dn = wp.tile([1, S], f32, tag="dn", bufs=1)
            nc.vector.tensor_copy(dn[:], pv[64:65, :])
            rc = wp.tile([1, S], f32, tag="rc", bufs=1)
            nc.vector.reciprocal_approx_fast(rc[:], dn[:])
            bc = wp.tile([128, S], f32, tag="bc", bufs=3)
            nc.gpsimd.partition_broadcast(bc[:], rc[:], channels=128)
            nc.vector.tensor_mul(dsl, dsl,
                                 bc[(h % 2) * 64:(h % 2) * 64 + 64, :])

        def head_full(q2t, k2t, hh, h, inject=None):
            """Full 8-kt attention pipeline for one head. `inject(kt)`
            emits interleaved work (next block's projection matmuls) so
            ACT never waits for a projection burst at group boundaries."""
            pv = psm.tile([65, S], f32, tag="pv", bufs=1, name="pv")
            window = [emit_scores(q2t, k2t, hh, 0), emit_scores(q2t, k2t, hh, 1)]
            for kt in range(NT):
                pss = window.pop(0)
                emit_exp_pv(h, kt, pss, pv)
                if kt + 2 < NT:
                    window.append(emit_scores(q2t, k2t, hh, kt + 2))
                if inject is not None:
                    inject(kt)
            emit_norm(h, pv)

        # ---- fused pipeline. Heads 2g/2g+1 run while block g+1's
        # projection matmuls trickle through their kt loops (2 per step),
        # so softplus input is ready the moment the head finishes and the
        # ACT queue never drains at a group boundary.
        for st in range(NT):
            proj_v(st)
        # wout prefetch: rotates into wv's slot, needed ~200us later
        wo = load_w_out()

        ps = proj_ps()
        for di in range(ND):
            proj_mm(False, 0, ps, di)
        q2p = proj_finish(False, 0, ps)
        ps = proj_ps()
        for di in range(ND):
            proj_mm(True, 0, ps, di)
        k2p = proj_finish(True, 0, ps)

        for g in range(8):
            if g < 7:
                psq = proj_ps()
                head_full(q2p, k2p, 0, 2 * g,
                          inject=lambda kt: proj_mm(False, g + 1, psq, kt))
                q2n = proj_finish(False, g + 1, psq)
                psk = proj_ps()
                head_full(q2p, k2p, 1, 2 * g + 1,
                          inject=lambda kt: proj_mm(True, g + 1, psk, kt))
                k2n = proj_finish(True, g + 1, psk)
                q2p, k2p = q2n, k2n
            else:
                head_full(q2p, k2p, 0, 14)
                head_full(q2p, k2p, 1, 15)

        psm_cm.__exit__(None, None, None)

        # ---------------- output projection ----------------
        with tc.tile_pool(name="psum_tail", bufs=1, space="PSUM") as pst:
            for st in range(NT):
                ps = pst.tile([128, D], f32, tag="ps_o", bufs=2)
                last = st == NT - 1
                for half in range(2):
                    lo = half * 512
                    for et in range(ND):
                        lhsT = attnT[:, et * S + st * 128:et * S + (st + 1) * 128]
                        rhs = wo[:, et * D + lo:et * D + lo + 512]
                        nc.tensor.matmul(ps[:, lo:lo + 512], lhsT, rhs,
                                         start=(et == 0), stop=(et == ND - 1))
                    if last:
                        # final tile: evacuate each half as soon as its
                        # accumulation lands, halving the serial exit chain
                        ot = wp.tile([128, 512], f32, tag="out_hb", bufs=2)
                        nc.vector.tensor_copy(ot[:], ps[:, lo:lo + 512])
                        nc.sync.dma_start(
                            t_out[st * 128:(st + 1) * 128, lo:lo + 512], ot[:])
                if not last:
                    ot = wp.tile([128, D], f32, tag="out_sb", bufs=2)
                    nc.vector.tensor_copy(ot[:], ps[:])
                    nc.sync.dma_start(t_out[st * 128:(st + 1) * 128, :], ot[:])


def build_bass(reps=1, **emit_kw):
    import concourse.bass as bass  # noqa: F401
    import concourse.mybir as mybir
    import concourse.tile as tile
    from concourse import bacc

    dt = mybir.dt
    nc = bacc.Bacc("TRN2", target_bir_lowering=False, debug=False)
    t_hsT = nc.dram_tensor("hsT", [D, S], dt.bfloat16, kind="ExternalInput").ap()
    t_wqkvT = nc.dram_tensor("wqkvT", [D, 3 * D], dt.bfloat16, kind="ExternalInput").ap()
    t_woutT = nc.dram_tensor("woutT", [D, D], dt.bfloat16, kind="ExternalInput").ap()
    t_rot = nc.dram_tensor("rot", [128, 4 * S], dt.bfloat16, kind="ExternalInput").ap()
    t_out = nc.dram_tensor("out", [S, D], dt.float32, kind="ExternalOutput").ap()
    with tile.TileContext(nc) as tc:
        if reps == 1:
            _emit(tc, nc, t_hsT, t_wqkvT, t_woutT, t_rot, t_out, **emit_kw)
        else:
            with tc.For_i(0, reps, 1):
                _emit(tc, nc, t_hsT, t_wqkvT, t_woutT, t_rot, t_out, **emit_kw)
    nc.compile()
    return nc


def make_in_maps(hidden_states, pos_xyz, w_qkv, w_out, phase_bias):
    hsT, wqkvT, woutT, cq, sq, ck, sk = _host_prep(
        hidden_states, pos_xyz, w_qkv, w_out, phase_bias)
    rot = np.concatenate([cq, sq, ck, sk], axis=2)  # [B, 128, 4*S]
    return [
        {
            "hsT": np.ascontiguousarray(hsT[b]),
            "wqkvT": wqkvT,
            "woutT": woutT,
            "rot": np.ascontiguousarray(rot[b]),
        }
        for b in range(B)
    ]


def kernel(hidden_states, attention_mask, pos_xyz, w_qkv, w_out, phase_bias):
    from concourse.bass_utils import run_bass_kernel_spmd

    in_maps = make_in_maps(hidden_states, pos_xyz, w_qkv, w_out, phase_bias)
    nc = build_bass()
    res = run_bass_kernel_spmd(nc, in_maps, list(range(B)))
    out = np.stack([np.asarray(res.results[c]["out"]) for c in range(B)])
    return out.astype(np.float32)
